# revision 21
# baseline (speedup 1.0000x reference)
"""Trainium2 Bass kernel for nn_Net_2491081031714.

Math per row x (784 f32):
  s_k = sum_{j>=k} x_j^2, theta_k = arccos(x_k/sqrt(s_k)) (k=0..8),
  th_k = max(theta_k + A_k, B_k) with A = rot1+rot2+rot3, B = max(rot2,0)+rot3
  (relu-chain folding), cart = polar_to_cartesian(c*sqrt(s_0), th),
  out = softmax(cart).

theta_k is within +-0.2 of pi/2 for this input scale (|x_k| << sqrt(s_k)),
so each k's max() branch is constant across rows (host-verified with
margin).  With cos(theta)=x_k/sqrt(s_k) and sin(theta)=sqrt(s_{k+1}/s_k),
every sin(th_k)/cos(th_k) is algebraic, and writing
  sin(th_i) = sqrt(s_{i+1}/s_i) * ghat_i
makes the sqrt ratios telescope through the cumprod:
  logit_m = state_m * h_m,  state_m = c * prod_{i<m} ghat_i (one scan)
  ghat_i = ca_i + sa_i*x_i*rsqrt(s_{i+1})     [branch1]
         = sb_i * sqrt(s_i)*rsqrt(s_{i+1})    [branch2 slot patch]
  h_m    = ca_m*x_m - sa_m*sqrt(s_{m+1})      [branch1]
         = cb_m * sqrt(s_m)                   [branch2 slot patch]
  c0 = state_8*h_8, c1 = state_9*sqrt(s_9), tail_j = state_{7-j}*h_{7-j}
No trig activations at all -> ACT only needs {square, exp}, which live in
one table set (exp_and_others): no mid-kernel table switch.

x streams in fp16 over two DMA queues (Sync HWDGE + GpSimd SWDGE) in
symmetric 1/2/2/2/1-tile groups; each 128x775 tile is squared+reduced by
DVE (STT+accum) or ACT (Square+accum), with the per-tile s9 landing
directly in slot 8 of the suffix-sum tile S.  rsqrt is a Quake seed +
one Halley step on [P,NT,9].

Sharding: pure batch data-parallel over 8 cores (2048 rows each).
"""

import numpy as np

import concourse.bacc as bacc
import concourse.tile as tile
from concourse import mybir
from concourse.bass_utils import run_bass_kernel_spmd

AF = mybir.ActivationFunctionType
OP = mybir.AluOpType
F32 = mybir.dt.float32
I32 = mybir.dt.int32
F16 = mybir.dt.float16

B, N = 16384, 784
NCORES = 8
ROWS = B // NCORES          # 2048
P = 128
NT = ROWS // P              # 16 row-tiles per core
K = 9                       # thetas that matter
NO = 10                     # output classes
W = N - K                   # 775 streamed cols per row

RSQRT_MAGIC = 0x5F3759DF    # Quake rsqrt seed constant

# pc (host-prepared params) column layout
PC_CA = 0                   # cos(A_k), k=0..8
PC_SA = PC_CA + K           # sin(A_k)
PC_SB = PC_SA + K           # sin(B_i) for the (up to 2) branch2 slots
PC_CB = PC_SB + 2           # cos(B_i) likewise
PC_W = PC_CB + 2

# DMA groups (start_tile, n_tiles, ring): ring 0 = Sync HWDGE,
# ring 2 = GpSimd SWDGE.  Symmetric striping so tiles land in index
# order; single-tile first/last groups shorten the ramp and the tail.
# pc/x9 go on the otherwise-idle Scalar HWDGE queue (their 128 tiny
# descriptors would stall a ring that also carries x).
# Ring 0 (Sync HWDGE) carries exactly the DVE-consumed tiles, ring 2
# (GpSimd SWDGE) the ACT-consumed ones, so each engine is paced by its
# own ring only.  No two groups within a ring are row-adjacent.
GROUPS = [(0, 1, 0), (1, 1, 2), (2, 2, 0), (4, 2, 2), (6, 2, 0),
          (8, 2, 2), (10, 2, 0), (12, 2, 2), (14, 1, 0), (15, 1, 2)]
# epilogue phases: H0 = tiles [0, HJ) runs while H1 tiles still stream
HJ = 8
DVE_SQ_H0 = (0, 2, 3, 6, 7)
ACT_SQ_H0 = (1, 4, 5, 8, 9)      # 8,9 land early; they belong to H1 data-wise
DVE_SQ_H1 = (10, 11, 14)
ACT_SQ_H1 = (12, 13, 15)


def _build(c, ca, sa, b2_patches):
    """b2_patches: list of (slot_i, sb_i, cb_i) for constant-branch ks."""
    nc = bacc.Bacc("TRN2", target_bir_lowering=False, debug=False)
    x = nc.dram_tensor("x", [ROWS, N], F16, kind="ExternalInput")
    x9 = nc.dram_tensor("x9", [ROWS, K], F32, kind="ExternalInput")
    pc = nc.dram_tensor("pc", [P, PC_W], F32, kind="ExternalInput")
    y = nc.dram_tensor("y", [ROWS, NO], F32, kind="ExternalOutput")

    # row <-> (partition, slot) mapping: row = NT*p + t
    xg_view = x.rearrange("(p t) n -> p t n", p=P)              # [P, NT, N]
    x9_view = x9.rearrange("(p t) k -> p t k", p=P)             # [P, NT, K]
    y_view = y.rearrange("(p t) k -> p t k", p=P)               # [P, NT, NO]

    with tile.TileContext(nc) as tc:
        with (
            tc.tile_pool(name="xpool", bufs=1) as xpool,
            tc.tile_pool(name="sing", bufs=1) as sing,
        ):
            xg = [xpool.tile([P, nt, N], F16, name=f"xg{g}", tag=f"xg{g}")
                  for g, (t0, nt, ring) in enumerate(GROUPS)]
            pct = sing.tile([P, PC_W], F32)
            x9n = sing.tile([P, NT, K], F32)

            # pc/x9 on the Scalar HWDGE queue (ACT is idle pre-data)
            nc.scalar.dma_start(pct[:], pc[:])
            nc.scalar.dma_start(x9n[:], x9_view)
            ring_eng = {0: nc.sync, 2: nc.gpsimd}
            for g, (t0, nt, ring) in enumerate(GROUPS):
                ring_eng[ring].dma_start(xg[g][:], xg_view[:, t0:t0 + nt, :])

            # force the {square, exp} table set to load before data lands
            warm = sing.tile([P, 1], F32)
            nc.vector.memset(warm[:], 0.0)
            nc.scalar.activation(warm[:], warm[:], AF.Exp)

            # persistent tiles
            S = sing.tile([P, NT, K], F32)        # S_j = s_{j+1}; slot 8 = s9
            qsr = sing.tile([P, NT, K - 1], F32)  # slot m: q_{8-m}
            sq9r = sing.tile([P, NT, K - 1], F32)  # slot m: x_{8-m}^2
            g01 = sing.tile([P, NT, K - 1], F32)  # per-block scan gate
            xt = sing.tile([P, NT, K], F32)       # sa_i * x_i
            xh2 = sing.tile([P, NT, K], F32)      # ca_m * x_m
            scanD = sing.tile([P, NT, NO], F32)   # [gate0, ghat_0..ghat_8]
            zc = sing.tile([P, NT, NO], F32)      # [c, 0...0] scan data1
            sqA = sing.tile([P, W], F16)          # DVE STT dead out
            sqC = sing.tile([P, W], F16)          # ACT square dead out

            # ---- in-stream preps on gpsimd (need only pc/x9) ----
            nc.gpsimd.memset(g01[:], 1.0)
            nc.gpsimd.memset(g01[:, :, 0:1], 0.0)       # block-start gate
            nc.gpsimd.memset(zc[:], 0.0)
            nc.gpsimd.memset(zc[:, :, 0:1], float(c))
            nc.gpsimd.memset(scanD[:, :, 0:1], 0.0)     # gate slot
            cav = pct[:, PC_CA:PC_CA + K].unsqueeze(1).broadcast_to([P, NT, K])
            sav = pct[:, PC_SA:PC_SA + K].unsqueeze(1).broadcast_to([P, NT, K])
            nc.gpsimd.tensor_tensor(out=xt[:], in0=x9n[:], in1=sav, op=OP.mult)
            nc.gpsimd.tensor_tensor(out=xh2[:], in0=x9n[:], in1=cav,
                                    op=OP.mult)
            # sq9r slot m = x_{8-m}^2 (m=0..7; x_0 never enters any q)
            nc.gpsimd.tensor_mul(sq9r[:], x9n[:, :, K - 1:0:-1],
                                 x9n[:, :, K - 1:0:-1])
            # forward prefix scan -> qsr slot m = q_{8-m} = sum x_{8-m..8}^2
            # (scan is DVE-only; runs early, long before the epilogue)
            nc.vector.tensor_tensor_scan(
                out=qsr[:].rearrange("p b k -> p (b k)"),
                data0=g01[:].rearrange("p b k -> p (b k)"),
                data1=sq9r[:].rearrange("p b k -> p (b k)"),
                initial=0.0, op0=OP.mult, op1=OP.add,
            )

            # ---- streaming square+reduce; s9 lands in S slot 8 ----
            tile_group = {}
            for g, (t0, nt, ring) in enumerate(GROUPS):
                for j in range(nt):
                    tile_group[t0 + j] = (g, j)

            def emit_sq_dve(t):
                g, j = tile_group[t]
                nc.vector.scalar_tensor_tensor(
                    out=sqA[:], in0=xg[g][:, j, K:N], scalar=1.0,
                    in1=xg[g][:, j, K:N], op0=OP.mult, op1=OP.mult,
                    accum_out=S[:, t, K - 1:K],
                )

            def emit_sq_act(t):
                g, j = tile_group[t]
                nc.scalar.activation(out=sqC[:], in_=xg[g][:, j, K:N],
                                     func=AF.Square,
                                     accum_out=S[:, t, K - 1:K])

            # ---- epilogue tiles (full NT; chains run on half slices) ----
            ep = sing
            y0i = ep.tile([P, NT, K], I32)
            aa = ep.tile([P, NT, K], F32)
            ww = ep.tile([P, NT, K], F32)
            inv = ep.tile([P, NT, K], F32)
            root = ep.tile([P, NT, K], F32)       # root_j = sqrt(s_{j+1})
            t1 = ep.tile([P, NT, K], F32)
            t2 = ep.tile([P, NT, K], F32)
            h = ep.tile([P, NT, K], F32)
            ptt = ep.tile([P, NT, 2], F32)
            state = ep.tile([P, NT, NO], F32)
            cart = ep.tile([P, NT, NO], F32)
            E = ep.tile([P, NT, NO], F32)
            ds = ep.tile([P, NT], F32)
            dinv = ep.tile([P, NT], F32)
            out = ep.tile([P, NT, NO], F32)
            dacc = ep.tile([P, 1], F32)

            def emit_chain(h0, h1):
                hn = h1 - h0
                cavh = pct[:, PC_CA:PC_CA + K].unsqueeze(1).broadcast_to(
                    [P, hn, K])
                savh = pct[:, PC_SA:PC_SA + K].unsqueeze(1).broadcast_to(
                    [P, hn, K])
                # S_j = s9 + q_{j+1} for j=0..7 (slot 8 already holds s9)
                s9b = S[:, h0:h1, K - 1:K].broadcast_to([P, hn, K - 1])
                nc.vector.tensor_tensor(out=S[:, h0:h1, 0:K - 1],
                                        in0=qsr[:, h0:h1, ::-1], in1=s9b,
                                        op=OP.add)
                # Quake rsqrt + one Halley step: inv_j = rsqrt(s_{j+1})
                sbits = S[:, h0:h1, :].bitcast(I32)
                nc.vector.tensor_scalar(out=y0i[:, h0:h1, :], in0=sbits,
                                        scalar1=1, scalar2=-1,
                                        op0=OP.arith_shift_right,
                                        op1=OP.bitwise_xor)
                nc.vector.tensor_scalar(out=y0i[:, h0:h1, :],
                                        in0=y0i[:, h0:h1, :],
                                        scalar1=RSQRT_MAGIC + 1, scalar2=None,
                                        op0=OP.add)
                yv = y0i[:, h0:h1, :].bitcast(F32)
                nc.vector.tensor_mul(aa[:, h0:h1, :], yv, yv)
                nc.vector.tensor_mul(ww[:, h0:h1, :], aa[:, h0:h1, :],
                                     S[:, h0:h1, :])
                nc.vector.affine_mul_reduce(out=aa[:, h0:h1, :],
                                            accum_out=dacc[:],
                                            in0=ww[:, h0:h1, :],
                                            in1=ww[:, h0:h1, :], scale=0.375,
                                            bias=-1.25)
                nc.vector.affine_mul_reduce(out=inv[:, h0:h1, :],
                                            accum_out=dacc[:],
                                            in0=aa[:, h0:h1, :], in1=yv,
                                            scale=1.0, bias=1.875)
                nc.vector.tensor_mul(root[:, h0:h1, :], S[:, h0:h1, :],
                                     inv[:, h0:h1, :])
                # ghat (branch1 form) into scanD slots 1..9
                nc.vector.tensor_mul(t1[:, h0:h1, :], xt[:, h0:h1, :],
                                     inv[:, h0:h1, :])
                nc.vector.tensor_tensor(out=scanD[:, h0:h1, 1:NO],
                                        in0=t1[:, h0:h1, :], in1=cavh,
                                        op=OP.add)
                # h (branch1 form) on gpsimd, in parallel with the DVE
                # scan path (joins again at the tail mul)
                nc.gpsimd.tensor_tensor(out=t2[:, h0:h1, :], in0=savh,
                                        in1=root[:, h0:h1, :], op=OP.mult)
                nc.gpsimd.tensor_tensor(out=h[:, h0:h1, :],
                                        in0=xh2[:, h0:h1, :],
                                        in1=t2[:, h0:h1, :], op=OP.subtract)
                # branch2 slot patches (tiny strided ops)
                if len(b2_patches) == 2:
                    i0 = b2_patches[0][0]
                    i1 = b2_patches[1][0]
                    st = i1 - i0
                    sbvv = pct[:, PC_SB:PC_SB + 2].unsqueeze(1).broadcast_to(
                        [P, hn, 2])
                    cbvv = pct[:, PC_CB:PC_CB + 2].unsqueeze(1).broadcast_to(
                        [P, hn, 2])
                    nc.vector.tensor_mul(ptt[:, h0:h1, :],
                                         root[:, h0:h1, i0 - 1:i1:st],
                                         inv[:, h0:h1, i0:i1 + 1:st])
                    nc.vector.tensor_tensor(
                        out=scanD[:, h0:h1, 1 + i0:2 + i1:st],
                        in0=ptt[:, h0:h1, :], in1=sbvv, op=OP.mult)
                    nc.gpsimd.tensor_tensor(
                        out=h[:, h0:h1, i0:i1 + 1:st],
                        in0=root[:, h0:h1, i0 - 1:i1:st],
                        in1=cbvv, op=OP.mult)
                else:
                    for (i, sb_i, cb_i) in b2_patches:
                        pt = ep.tile([P, NT, 1], F32, name=f"pt{i}_{h0}")
                        nc.vector.tensor_mul(pt[:, h0:h1, :],
                                             root[:, h0:h1, i - 1:i],
                                             inv[:, h0:h1, i:i + 1])
                        nc.vector.tensor_scalar(
                            out=scanD[:, h0:h1, 1 + i:2 + i],
                            in0=pt[:, h0:h1, :], scalar1=float(sb_i),
                            scalar2=None, op0=OP.mult)
                        nc.gpsimd.tensor_scalar(
                            out=h[:, h0:h1, i:i + 1],
                            in0=root[:, h0:h1, i - 1:i],
                            scalar1=float(cb_i), scalar2=None, op0=OP.mult)
                # state_m = c * prod_{i<m} ghat_i (data1 seeds c at slot 0)
                nc.vector.tensor_tensor_scan(
                    out=state[:, h0:h1, :].rearrange("p b k -> p (b k)"),
                    data0=scanD[:, h0:h1, :].rearrange("p b k -> p (b k)"),
                    data1=zc[:, h0:h1, :].rearrange("p b k -> p (b k)"),
                    initial=0.0, op0=OP.mult, op1=OP.add,
                )
                # tail: cart[2+j] = state_{7-j} * h_{7-j}
                nc.vector.tensor_mul(cart[:, h0:h1, 2:NO],
                                     state[:, h0:h1, 7::-1],
                                     h[:, h0:h1, 7::-1])
                nc.vector.tensor_mul(cart[:, h0:h1, 0:1],
                                     state[:, h0:h1, 8:9], h[:, h0:h1, 8:9])
                nc.vector.tensor_mul(cart[:, h0:h1, 1:2],
                                     state[:, h0:h1, 9:NO],
                                     root[:, h0:h1, 8:9])

            def emit_softmax(h0, h1):
                hn = h1 - h0
                nc.scalar.activation(E[:, h0:h1, :], cart[:, h0:h1, :], AF.Exp)
                nc.vector.tensor_reduce(out=ds[:, h0:h1], in_=E[:, h0:h1, :],
                                        axis=mybir.AxisListType.X, op=OP.add)
                nc.vector.reciprocal_approx_fast(dinv[:, h0:h1], ds[:, h0:h1])
                nc.vector.tensor_mul(
                    out[:, h0:h1, :], E[:, h0:h1, :],
                    dinv[:, h0:h1].unsqueeze(2).broadcast_to([P, hn, NO]))
                nc.sync.dma_start(y_view[:, h0:h1, :], out[:, h0:h1, :])

            # squares (engine order = land order), then one full-batch chain
            for t in DVE_SQ_H0 + DVE_SQ_H1:
                emit_sq_dve(t)
            for t in ACT_SQ_H0 + ACT_SQ_H1:
                emit_sq_act(t)
            emit_chain(0, NT)
            emit_softmax(0, HJ)
            emit_softmax(HJ, NT)

    nc.compile()
    return nc


_NC = None


def _host_params(scale1, rot1, scale2, rot2, scale3, rot3):
    c = max(max(float(scale1[0]), 0.0) * float(scale2[0]), 0.0) * float(scale3[0])
    r1 = rot1[:K].astype(np.float64)
    r2 = rot2[:K].astype(np.float64)
    r3 = rot3[:K].astype(np.float64)
    A = r1 + r2 + r3
    Bc = np.maximum(r2, 0.0) + r3
    D = Bc - A
    # theta_k = arccos(x_k/sqrt(s_k)) stays within pi/2 +- 0.25 for this
    # input scale; each max() branch must be constant across rows
    lo, hi = np.pi / 2 - 0.25, np.pi / 2 + 0.25
    branch2 = D >= hi
    branch1 = D <= lo
    assert np.all(branch1 | branch2), f"mixed max() branch: D={D}"
    assert not branch2[0], "constant branch at k=0 unsupported"
    ca = np.cos(A)
    sa = np.sin(A)
    patches = [(int(i), float(np.sin(Bc[i])), float(np.cos(Bc[i])))
               for i in np.nonzero(branch2)[0]]
    assert len(patches) <= 2, "more than 2 constant-branch slots unsupported"
    row = np.zeros((PC_W,), np.float64)
    row[PC_CA:PC_CA + K] = ca
    row[PC_SA:PC_SA + K] = sa
    for n, (i, sb_i, cb_i) in enumerate(patches):
        row[PC_SB + n] = sb_i
        row[PC_CB + n] = cb_i
    pc = np.tile(row.astype(np.float32)[None, :], (P, 1))
    return c, pc, patches


def kernel(x, scale1, rot1, scale2, rot2, scale3, rot3, _trace=False):
    global _NC
    c, pc, patches = _host_params(scale1, rot1, scale2, rot2, scale3, rot3)
    if _NC is None:
        ca = pc[0, PC_CA:PC_CA + K]
        sa = pc[0, PC_SA:PC_SA + K]
        _NC = _build(c, ca, sa, patches)
    nc = _NC
    x = np.ascontiguousarray(x, dtype=np.float32)
    xh = x.astype(np.float16)
    x9h = np.ascontiguousarray(x[:, 0:K])
    in_maps = [
        {"x": xh[cid * ROWS:(cid + 1) * ROWS], "pc": pc,
         "x9": x9h[cid * ROWS:(cid + 1) * ROWS]} for cid in range(NCORES)
    ]
    res = run_bass_kernel_spmd(nc, in_maps, core_ids=list(range(NCORES)),
                               trace=_trace)
    out = np.concatenate([res.results[cid]["y"] for cid in range(NCORES)],
                         axis=0)
    if _trace:
        return out, res
    return out


# revision 22
# speedup vs baseline: 1.0686x; 1.0686x over previous
"""Trainium2 Bass kernel for nn_Net_2491081031714.

Math per row x (784 f32):
  s_k = sum_{j>=k} x_j^2, theta_k = arccos(x_k/sqrt(s_k)) (k=0..8),
  th_k = max(theta_k + A_k, B_k) with A = rot1+rot2+rot3, B = max(rot2,0)+rot3
  (relu-chain folding), cart = polar_to_cartesian(c*sqrt(s_0), th),
  out = softmax(cart).

theta_k is within +-0.2 of pi/2 for this input scale (|x_k| << sqrt(s_k)),
so each k's max() branch is constant across rows (host-verified with
margin).  With cos(theta)=x_k/sqrt(s_k) and sin(theta)=sqrt(s_{k+1}/s_k),
every sin(th_k)/cos(th_k) is algebraic, and writing
  sin(th_i) = sqrt(s_{i+1}/s_i) * ghat_i
makes the sqrt ratios telescope through the cumprod:
  logit_m = state_m * h_m,  state_m = c * prod_{i<m} ghat_i (one scan)
  ghat_i = ca_i + sa_i*x_i*rsqrt(s_{i+1})     [branch1]
         = sb_i * sqrt(s_i)*rsqrt(s_{i+1})    [branch2 slot patch]
  h_m    = ca_m*x_m - sa_m*sqrt(s_{m+1})      [branch1]
         = cb_m * sqrt(s_m)                   [branch2 slot patch]
  c0 = state_8*h_8, c1 = state_9*sqrt(s_9), tail_j = state_{7-j}*h_{7-j}
No trig activations at all -> ACT only needs {square, exp}, which live in
one table set (exp_and_others): no mid-kernel table switch.

x streams in fp16 over two DMA queues (Sync HWDGE + GpSimd SWDGE) in
symmetric 1/2/2/2/1-tile groups; each 128x775 tile is squared+reduced by
DVE (STT+accum) or ACT (Square+accum), with the per-tile s9 landing
directly in slot 8 of the suffix-sum tile S.  rsqrt is a Quake seed +
one Halley step on [P,NT,9].

Sharding: pure batch data-parallel over 8 cores (2048 rows each).
"""

import numpy as np

import concourse.bacc as bacc
import concourse.tile as tile
from concourse import mybir
from concourse.bass_utils import run_bass_kernel_spmd

AF = mybir.ActivationFunctionType
OP = mybir.AluOpType
F32 = mybir.dt.float32
I32 = mybir.dt.int32
F16 = mybir.dt.float16

B, N = 16384, 784
NCORES = 8
ROWS = B // NCORES          # 2048
P = 128
NT = ROWS // P              # 16 row-tiles per core
K = 9                       # thetas that matter
NO = 10                     # output classes
W = N - K                   # 775 streamed cols per row

RSQRT_MAGIC = 0x5F3759DF    # Quake rsqrt seed constant

# pc (host-prepared params) column layout
PC_CA = 0                   # cos(A_k), k=0..8
PC_SA = PC_CA + K           # sin(A_k)
PC_SB = PC_SA + K           # sin(B_i) for the (up to 2) branch2 slots
PC_CB = PC_SB + 2           # cos(B_i) likewise
PC_W = PC_CB + 2

# DMA groups (start_tile, n_tiles, ring): ring 0 = Sync HWDGE,
# ring 2 = GpSimd SWDGE.  Symmetric striping so tiles land in index
# order; single-tile first/last groups shorten the ramp and the tail.
# pc/x9 go on the otherwise-idle Scalar HWDGE queue (their 128 tiny
# descriptors would stall a ring that also carries x).
# Ring 0 (Sync HWDGE) carries exactly the DVE-consumed tiles, ring 2
# (GpSimd SWDGE) the ACT-consumed ones, so each engine is paced by its
# own ring only.  No two groups within a ring are row-adjacent.
GROUPS = [(0, 1, 0), (1, 1, 2), (2, 2, 0), (4, 2, 2), (6, 2, 0),
          (8, 2, 2), (10, 2, 0), (12, 2, 2), (14, 1, 0), (15, 1, 2)]
# epilogue phases: H0 = tiles [0, HJ) runs while H1 tiles still stream
HJ = 8
DVE_SQ_H0 = (0, 2, 3, 6, 7)
ACT_SQ_H0 = (1, 4, 5, 8, 9)      # 8,9 land early; they belong to H1 data-wise
DVE_SQ_H1 = (10, 11, 14)
ACT_SQ_H1 = (12, 13, 15)


def _build(c, ca, sa, b2_patches):
    """b2_patches: list of (slot_i, sb_i, cb_i) for constant-branch ks."""
    nc = bacc.Bacc("TRN2", target_bir_lowering=False, debug=False)
    x = nc.dram_tensor("x", [ROWS, N], F16, kind="ExternalInput")
    x9 = nc.dram_tensor("x9", [ROWS, K], F32, kind="ExternalInput")
    pc = nc.dram_tensor("pc", [P, PC_W], F32, kind="ExternalInput")
    y = nc.dram_tensor("y", [ROWS, NO], F32, kind="ExternalOutput")

    # row <-> (partition, slot) mapping: row = NT*p + t
    xg_view = x.rearrange("(p t) n -> p t n", p=P)              # [P, NT, N]
    x9_view = x9.rearrange("(p t) k -> p t k", p=P)             # [P, NT, K]
    y_view = y.rearrange("(p t) k -> p t k", p=P)               # [P, NT, NO]

    with tile.TileContext(nc) as tc:
        with (
            tc.tile_pool(name="xpool", bufs=1) as xpool,
            tc.tile_pool(name="sing", bufs=1) as sing,
        ):
            xg = [xpool.tile([P, nt, N], F16, name=f"xg{g}", tag=f"xg{g}")
                  for g, (t0, nt, ring) in enumerate(GROUPS)]
            pct = sing.tile([P, PC_W], F32)
            x9n = sing.tile([P, NT, K], F32)

            # pc/x9 on the Scalar HWDGE queue (ACT is idle pre-data)
            nc.scalar.dma_start(pct[:], pc[:])
            nc.scalar.dma_start(x9n[:], x9_view)
            ring_eng = {0: nc.sync, 2: nc.gpsimd}
            for g, (t0, nt, ring) in enumerate(GROUPS):
                ring_eng[ring].dma_start(xg[g][:], xg_view[:, t0:t0 + nt, :])

            # force the {square, exp} table set to load before data lands
            warm = sing.tile([P, 1], F32)
            nc.vector.memset(warm[:], 0.0)
            nc.scalar.activation(warm[:], warm[:], AF.Exp)

            # persistent tiles
            S = sing.tile([P, NT, K], F32)        # S_j = s_{j+1}; slot 8 = s9
            qsr = sing.tile([P, NT, K - 1], F32)  # slot m: q_{8-m}
            sq9r = sing.tile([P, NT, K - 1], F32)  # slot m: x_{8-m}^2
            g01 = sing.tile([P, NT, K - 1], F32)  # per-block scan gate
            xt = sing.tile([P, NT, K], F32)       # sa_i * x_i
            xh2 = sing.tile([P, NT, K], F32)      # ca_m * x_m
            scanD = sing.tile([P, NT, NO], F32)   # [gate0, ghat_0..ghat_8]
            zc = sing.tile([P, NT, NO], F32)      # [c, 0...0] scan data1
            sqA = sing.tile([P, W], F16)          # DVE STT dead out
            sqC = sing.tile([P, W], F16)          # ACT square dead out

            # ---- in-stream preps on gpsimd (need only pc/x9) ----
            nc.gpsimd.memset(g01[:], 1.0)
            nc.gpsimd.memset(g01[:, :, 0:1], 0.0)       # block-start gate
            nc.gpsimd.memset(zc[:], 0.0)
            nc.gpsimd.memset(zc[:, :, 0:1], float(c))
            nc.gpsimd.memset(scanD[:, :, 0:1], 0.0)     # gate slot
            cav = pct[:, PC_CA:PC_CA + K].unsqueeze(1).broadcast_to([P, NT, K])
            sav = pct[:, PC_SA:PC_SA + K].unsqueeze(1).broadcast_to([P, NT, K])
            nc.gpsimd.tensor_tensor(out=xt[:], in0=x9n[:], in1=sav, op=OP.mult)
            nc.gpsimd.tensor_tensor(out=xh2[:], in0=x9n[:], in1=cav,
                                    op=OP.mult)
            # sq9r slot m = x_{8-m}^2 (m=0..7; x_0 never enters any q)
            nc.gpsimd.tensor_mul(sq9r[:], x9n[:, :, K - 1:0:-1],
                                 x9n[:, :, K - 1:0:-1])
            # forward prefix scan -> qsr slot m = q_{8-m} = sum x_{8-m..8}^2
            # (scan is DVE-only; runs early, long before the epilogue)
            nc.vector.tensor_tensor_scan(
                out=qsr[:].rearrange("p b k -> p (b k)"),
                data0=g01[:].rearrange("p b k -> p (b k)"),
                data1=sq9r[:].rearrange("p b k -> p (b k)"),
                initial=0.0, op0=OP.mult, op1=OP.add,
            )

            # ---- streaming square+reduce; s9 lands in S slot 8 ----
            tile_group = {}
            for g, (t0, nt, ring) in enumerate(GROUPS):
                for j in range(nt):
                    tile_group[t0 + j] = (g, j)

            def emit_sq_dve(t):
                g, j = tile_group[t]
                nc.vector.scalar_tensor_tensor(
                    out=sqA[:], in0=xg[g][:, j, K:N], scalar=1.0,
                    in1=xg[g][:, j, K:N], op0=OP.mult, op1=OP.mult,
                    accum_out=S[:, t, K - 1:K],
                )

            def emit_sq_act(t):
                g, j = tile_group[t]
                nc.scalar.activation(out=sqC[:], in_=xg[g][:, j, K:N],
                                     func=AF.Square,
                                     accum_out=S[:, t, K - 1:K])

            # ---- epilogue tiles (full NT; chains run on half slices) ----
            ep = sing
            y0i = ep.tile([P, NT, K], I32)
            aa = ep.tile([P, NT, K], F32)
            ww = ep.tile([P, NT, K], F32)
            inv = ep.tile([P, NT, K], F32)
            root = ep.tile([P, NT, K], F32)       # root_j = sqrt(s_{j+1})
            t1 = ep.tile([P, NT, K], F32)
            t2 = ep.tile([P, NT, K], F32)
            h = ep.tile([P, NT, K], F32)
            ptt = ep.tile([P, NT, 2], F32)
            state = ep.tile([P, NT, NO], F32)
            cart = ep.tile([P, NT, NO], F32)
            E = ep.tile([P, NT, NO], F32)
            ds = ep.tile([P, NT], F32)
            dinv = ep.tile([P, NT], F32)
            out = ep.tile([P, NT, NO], F32)
            dacc = ep.tile([P, 1], F32)

            def emit_chain(h0, h1):
                hn = h1 - h0
                cavh = pct[:, PC_CA:PC_CA + K].unsqueeze(1).broadcast_to(
                    [P, hn, K])
                savh = pct[:, PC_SA:PC_SA + K].unsqueeze(1).broadcast_to(
                    [P, hn, K])
                # S_j = s9 + q_{j+1} for j=0..7 (slot 8 already holds s9)
                s9b = S[:, h0:h1, K - 1:K].broadcast_to([P, hn, K - 1])
                nc.vector.tensor_tensor(out=S[:, h0:h1, 0:K - 1],
                                        in0=qsr[:, h0:h1, ::-1], in1=s9b,
                                        op=OP.add)
                # Quake rsqrt + one Halley step: inv_j = rsqrt(s_{j+1})
                sbits = S[:, h0:h1, :].bitcast(I32)
                nc.vector.tensor_scalar(out=y0i[:, h0:h1, :], in0=sbits,
                                        scalar1=1, scalar2=-1,
                                        op0=OP.arith_shift_right,
                                        op1=OP.bitwise_xor)
                nc.vector.tensor_scalar(out=y0i[:, h0:h1, :],
                                        in0=y0i[:, h0:h1, :],
                                        scalar1=RSQRT_MAGIC + 1, scalar2=None,
                                        op0=OP.add)
                yv = y0i[:, h0:h1, :].bitcast(F32)
                nc.vector.tensor_mul(aa[:, h0:h1, :], yv, yv)
                nc.vector.tensor_mul(ww[:, h0:h1, :], aa[:, h0:h1, :],
                                     S[:, h0:h1, :])
                nc.vector.affine_mul_reduce(out=aa[:, h0:h1, :],
                                            accum_out=dacc[:],
                                            in0=ww[:, h0:h1, :],
                                            in1=ww[:, h0:h1, :], scale=0.375,
                                            bias=-1.25)
                nc.vector.affine_mul_reduce(out=inv[:, h0:h1, :],
                                            accum_out=dacc[:],
                                            in0=aa[:, h0:h1, :], in1=yv,
                                            scale=1.0, bias=1.875)
                nc.vector.tensor_mul(root[:, h0:h1, :], S[:, h0:h1, :],
                                     inv[:, h0:h1, :])
                # ghat (branch1 form) into scanD slots 1..9
                nc.vector.tensor_mul(t1[:, h0:h1, :], xt[:, h0:h1, :],
                                     inv[:, h0:h1, :])
                nc.vector.tensor_tensor(out=scanD[:, h0:h1, 1:NO],
                                        in0=t1[:, h0:h1, :], in1=cavh,
                                        op=OP.add)
                # h (branch1 form) on gpsimd, in parallel with the DVE
                # scan path (joins again at the tail mul)
                nc.gpsimd.tensor_tensor(out=t2[:, h0:h1, :], in0=savh,
                                        in1=root[:, h0:h1, :], op=OP.mult)
                nc.gpsimd.tensor_tensor(out=h[:, h0:h1, :],
                                        in0=xh2[:, h0:h1, :],
                                        in1=t2[:, h0:h1, :], op=OP.subtract)
                # branch2 slot patches (tiny strided ops)
                if len(b2_patches) == 2:
                    i0 = b2_patches[0][0]
                    i1 = b2_patches[1][0]
                    st = i1 - i0
                    sbvv = pct[:, PC_SB:PC_SB + 2].unsqueeze(1).broadcast_to(
                        [P, hn, 2])
                    cbvv = pct[:, PC_CB:PC_CB + 2].unsqueeze(1).broadcast_to(
                        [P, hn, 2])
                    nc.vector.tensor_mul(ptt[:, h0:h1, :],
                                         root[:, h0:h1, i0 - 1:i1:st],
                                         inv[:, h0:h1, i0:i1 + 1:st])
                    nc.vector.tensor_tensor(
                        out=scanD[:, h0:h1, 1 + i0:2 + i1:st],
                        in0=ptt[:, h0:h1, :], in1=sbvv, op=OP.mult)
                    nc.gpsimd.tensor_tensor(
                        out=h[:, h0:h1, i0:i1 + 1:st],
                        in0=root[:, h0:h1, i0 - 1:i1:st],
                        in1=cbvv, op=OP.mult)
                else:
                    for (i, sb_i, cb_i) in b2_patches:
                        pt = ep.tile([P, NT, 1], F32, name=f"pt{i}_{h0}")
                        nc.vector.tensor_mul(pt[:, h0:h1, :],
                                             root[:, h0:h1, i - 1:i],
                                             inv[:, h0:h1, i:i + 1])
                        nc.vector.tensor_scalar(
                            out=scanD[:, h0:h1, 1 + i:2 + i],
                            in0=pt[:, h0:h1, :], scalar1=float(sb_i),
                            scalar2=None, op0=OP.mult)
                        nc.gpsimd.tensor_scalar(
                            out=h[:, h0:h1, i:i + 1],
                            in0=root[:, h0:h1, i - 1:i],
                            scalar1=float(cb_i), scalar2=None, op0=OP.mult)
                # state_m = c * prod_{i<m} ghat_i (data1 seeds c at slot 0)
                nc.vector.tensor_tensor_scan(
                    out=state[:, h0:h1, :].rearrange("p b k -> p (b k)"),
                    data0=scanD[:, h0:h1, :].rearrange("p b k -> p (b k)"),
                    data1=zc[:, h0:h1, :].rearrange("p b k -> p (b k)"),
                    initial=0.0, op0=OP.mult, op1=OP.add,
                )
                # tail: cart[2+j] = state_{7-j} * h_{7-j}
                nc.vector.tensor_mul(cart[:, h0:h1, 2:NO],
                                     state[:, h0:h1, 7::-1],
                                     h[:, h0:h1, 7::-1])
                nc.vector.tensor_mul(cart[:, h0:h1, 0:1],
                                     state[:, h0:h1, 8:9], h[:, h0:h1, 8:9])
                nc.vector.tensor_mul(cart[:, h0:h1, 1:2],
                                     state[:, h0:h1, 9:NO],
                                     root[:, h0:h1, 8:9])

            def emit_softmax(h0, h1):
                hn = h1 - h0
                nc.scalar.activation(E[:, h0:h1, :], cart[:, h0:h1, :], AF.Exp)
                nc.vector.tensor_reduce(out=ds[:, h0:h1], in_=E[:, h0:h1, :],
                                        axis=mybir.AxisListType.X, op=OP.add)
                nc.vector.reciprocal_approx_fast(dinv[:, h0:h1], ds[:, h0:h1])
                nc.vector.tensor_mul(
                    out[:, h0:h1, :], E[:, h0:h1, :],
                    dinv[:, h0:h1].unsqueeze(2).broadcast_to([P, hn, NO]))
                nc.sync.dma_start(y_view[:, h0:h1, :], out[:, h0:h1, :])

            # phase 0: early squares, then the H0 chain while H1 streams
            for t in DVE_SQ_H0:
                emit_sq_dve(t)
            for t in ACT_SQ_H0:
                emit_sq_act(t)
            emit_chain(0, HJ)
            # phase 1: late squares, H1 chain, then both softmaxes
            for t in DVE_SQ_H1:
                emit_sq_dve(t)
            for t in ACT_SQ_H1:
                emit_sq_act(t)
            emit_softmax(0, HJ)
            emit_chain(HJ, NT)
            emit_softmax(HJ, NT)

    nc.compile()
    return nc


_NC = None


def _host_params(scale1, rot1, scale2, rot2, scale3, rot3):
    c = max(max(float(scale1[0]), 0.0) * float(scale2[0]), 0.0) * float(scale3[0])
    r1 = rot1[:K].astype(np.float64)
    r2 = rot2[:K].astype(np.float64)
    r3 = rot3[:K].astype(np.float64)
    A = r1 + r2 + r3
    Bc = np.maximum(r2, 0.0) + r3
    D = Bc - A
    # theta_k = arccos(x_k/sqrt(s_k)) stays within pi/2 +- 0.25 for this
    # input scale; each max() branch must be constant across rows
    lo, hi = np.pi / 2 - 0.25, np.pi / 2 + 0.25
    branch2 = D >= hi
    branch1 = D <= lo
    assert np.all(branch1 | branch2), f"mixed max() branch: D={D}"
    assert not branch2[0], "constant branch at k=0 unsupported"
    ca = np.cos(A)
    sa = np.sin(A)
    patches = [(int(i), float(np.sin(Bc[i])), float(np.cos(Bc[i])))
               for i in np.nonzero(branch2)[0]]
    assert len(patches) <= 2, "more than 2 constant-branch slots unsupported"
    row = np.zeros((PC_W,), np.float64)
    row[PC_CA:PC_CA + K] = ca
    row[PC_SA:PC_SA + K] = sa
    for n, (i, sb_i, cb_i) in enumerate(patches):
        row[PC_SB + n] = sb_i
        row[PC_CB + n] = cb_i
    pc = np.tile(row.astype(np.float32)[None, :], (P, 1))
    return c, pc, patches


def kernel(x, scale1, rot1, scale2, rot2, scale3, rot3, _trace=False):
    global _NC
    c, pc, patches = _host_params(scale1, rot1, scale2, rot2, scale3, rot3)
    if _NC is None:
        ca = pc[0, PC_CA:PC_CA + K]
        sa = pc[0, PC_SA:PC_SA + K]
        _NC = _build(c, ca, sa, patches)
    nc = _NC
    x = np.ascontiguousarray(x, dtype=np.float32)
    xh = x.astype(np.float16)
    x9h = np.ascontiguousarray(x[:, 0:K])
    in_maps = [
        {"x": xh[cid * ROWS:(cid + 1) * ROWS], "pc": pc,
         "x9": x9h[cid * ROWS:(cid + 1) * ROWS]} for cid in range(NCORES)
    ]
    res = run_bass_kernel_spmd(nc, in_maps, core_ids=list(range(NCORES)),
                               trace=_trace)
    out = np.concatenate([res.results[cid]["y"] for cid in range(NCORES)],
                         axis=0)
    if _trace:
        return out, res
    return out
